# revision 8
# baseline (speedup 1.0000x reference)
"""BiMapGeo forward on 8 NeuronCores (TRN2, Bass/Tile).

P[b,o] = sum_c W[o,c]^T @ x[b,c] @ W[o,c]
  x: (256, 8, 128, 128) fp32 (symmetric in last two dims)
  W: (16, 8, 128, 64) fp32
  P: (256, 16, 64, 64) fp32

Sharding: data-parallel over batch (32 per core), W replicated.

Host-side prep (outside the NEFF, not on the HW critical path):
  - cast x, W to bf16 (error ~3e-3 << 2e-2 budget)
  - pre-transpose x to [NG, NI(j), BG, HI, NI(i)] per core so group loads are
    256 KB DMAs with 2 KB-contiguous per-partition runs
  - P comes back in a device-friendly layout [NG, NI(ph,p), WV, T, BG, NO],
    host inverse-permutes to [B, HO, NO, NO]

Per-core kernel structure (per group of BG=4 batches):
  mm1: M1[b,c] = x[b,c] @ Wc           (bf16, stationary=x[b,c] via symmetry,
                                        moving=W[:,c] as [128, 1024] in 2x512,
                                        each 512-half into its own PSUM bank)
  evict: PSUM fp32 -> SBUF bf16        (h0 half on VectorE, h1 on ScalarE,
                                        concurrently on different banks)
  mm2: P[b,o] += W[o,c]^T @ M1[b,o,c]  (bf16, col-tiled o-pairs, N=256,
                                        accumulate over c in PSUM fp32)
  wave A: o-pairs 0..3 accumulate during the mm1/c loop (software pipelined)
  wave B: o-pairs 4..7 as PE tail, interleaved with the NEXT group's c=0
          mm1 phase so its evictions are latency-hidden
  evict P -> SBUF group tile -> one 1 MB DMA per group
"""

import numpy as np
from contextlib import ExitStack

import concourse.bacc as bacc
import concourse.tile as tile
from concourse import mybir

B_TOT, HI, HO, NI, NO = 256, 8, 16, 128, 64
NCORES = 8
B = B_TOT // NCORES  # 32 batches per core
BG = 4               # batches per group
NG = B // BG         # 8 groups
OQ = HO * NO         # 1024
WV = 4               # eviction waves per group (4 o's each)

F32 = mybir.dt.float32
BF16 = mybir.dt.bfloat16

_NC_CACHE = {}


def build_nc(loop_iters: int = 1):
    nc = bacc.Bacc("TRN2", target_bir_lowering=False, debug=False)

    x_in = nc.dram_tensor("x", [NG, NI, BG, HI, NI], BF16, kind="ExternalInput")
    w_in = nc.dram_tensor("W", [NI, HI, HO, NO], BF16, kind="ExternalInput")
    p_out = nc.dram_tensor("P", [NG, NI, WV, 2, BG, NO], F32, kind="ExternalOutput")

    with tile.TileContext(nc) as tc, ExitStack() as ctx:
        const = ctx.enter_context(tc.tile_pool(name="const", bufs=1))
        xpool = ctx.enter_context(tc.tile_pool(name="xpool", bufs=2))
        m1pool = ctx.enter_context(tc.tile_pool(name="m1pool", bufs=11))
        ppool = ctx.enter_context(tc.tile_pool(name="ppool", bufs=2))
        m1ps_pool = ctx.enter_context(tc.tile_pool(name="m1ps", bufs=4, space="PSUM"))
        pps_pool = ctx.enter_context(tc.tile_pool(name="pps", bufs=4, space="PSUM"))

        # W resident in SBUF as [i(128), c, o, q] bf16; serves both as mm1
        # moving operand (W[j, oq] per c) and mm2 stationary (W[i, p] per o,c).
        # Per-c DMAs so the first mm1 can start after one slice lands.
        w_sb = const.tile([NI, HI, HO, NO], BF16, tag="w_sb")
        for c in range(HI):
            nc.scalar.dma_start(out=w_sb[:, c], in_=w_in[:, c])

        def emit_body():
            emit_groups(nc, tc, x_in, p_out, w_sb, xpool, m1pool, ppool, m1ps_pool, pps_pool)

        if loop_iters > 1:
            ET = mybir.EngineType
            with tc.For_i(0, loop_iters, 1, hint_engines=(ET.PE, ET.DVE, ET.Activation, ET.SP)):
                emit_body()
        else:
            emit_body()
    nc.finalize()
    return nc


def emit_groups(nc, tc, x_in, p_out, w_sb, xpool, m1pool, ppool, m1ps_pool, pps_pool):
    # pps tiles are a full PSUM bank ([128, 512] f32 = 2 KB/partition); only
    # cols 0..255 are used. Guarantees one accumulation group per bank
    # (matmul start=1 clears has_written for the whole bank) and no
    # PE-write/engine-read same-bank overlap across waves.
    def pps_alloc(name):
        return pps_pool.tile([NI, 512], F32, tag="pps", name=name)

    def mm2(pps_t, c, o, ph, m1_c):
        # two M=32 col-tiled quads per o: 4 consecutive quads in a wave slot
        # hit positions {0,32,64,96} and stream 4-way concurrently on HW
        for h in range(2):
            nc.tensor.matmul(
                pps_t[ph * 64 + 32 * h : ph * 64 + 32 * h + 32, 0 : BG * NO],
                w_sb[:, c, o, 32 * h : 32 * h + 32],
                m1_c[:, :, o * 64 : (o + 1) * 64],
                start=(c == 0),
                stop=(c == HI - 1),
                tile_position=(0, ph * 64 + 32 * h),
                skip_group_check=True,
            )

    # one PSUM pair (o-pair 2wv, 2wv+1) -> slice of the group P tile
    def evict_wave(wv, pps_pair, p_g):
        for t in range(2):
            if (wv + t) % 2 == 0:
                nc.scalar.copy(p_g[:, wv, t], pps_pair[t][:, 0 : BG * NO])
            else:
                nc.vector.tensor_copy(p_g[:, wv, t], pps_pair[t][:, 0 : BG * NO])

    def x_load(g):
        x_t = xpool.tile([NI, BG, HI, NI], BF16, tag="xt", name=f"xt{g}")
        for b in range(BG):
            nc.sync.dma_start(out=x_t[:, b], in_=x_in[g, :, b])
        return x_t

    # mm1 for one (c, b): two 512-wide halves into separate single-bank PSUM
    # tiles, evicted concurrently (h0 on DVE, h1 on ACT; every 8th h0 goes to
    # the faster ACT to balance 0.96 vs 1.2 GHz engine rates: DVE 28/36 ACT).
    def mm1_b(x_t, m1_c, c, b):
        for h in range(2):
            m1_ps = m1ps_pool.tile([NI, 512], F32, tag="m1ps")
            nc.tensor.matmul(
                m1_ps[:],
                x_t[:, b, c, :],
                w_sb[:, c, h * 8 : (h + 1) * 8, :],
                start=True,
                stop=True,
            )
            if h == 0 and (c * BG + b) % 8 != 7:
                nc.vector.tensor_copy(m1_c[:, b, 0:512], m1_ps[:])
            else:
                nc.scalar.copy(m1_c[:, b, h * 512 : h * 512 + 512], m1_ps[:])

    x_tiles = {0: x_load(0)}
    # carry: group g's c=0 m1 tile, produced inside g-1's wave-B section
    m1_c0_carry = None

    for g in range(NG):
        if g + 1 < NG:
            x_tiles[g + 1] = x_load(g + 1)
        x_t = x_tiles.pop(g)

        # group P staging tile: [128(ph,p), wv, t, b, q] fp32
        p_g = ppool.tile([NI, WV, 2, BG, NO], F32, tag="pg", name=f"pg{g}")

        # wave A: o-pairs 0..3 PSUM accumulators, held across the c loop
        ppsA = [pps_alloc(f"ppsA_g{g}t{t}") for t in range(4)]

        if m1_c0_carry is not None:
            m1_tiles = [m1_c0_carry]
        else:
            # first group of the body: bare c=0 phase
            m1_c = m1pool.tile([NI, BG, OQ], BF16, tag="m1")
            for b in range(BG):
                mm1_b(x_t, m1_c, 0, b)
            m1_tiles = [m1_c]

        # c=1..7: mm1 + eviction + wave-A mm2 of c-1, pipelined
        for c in range(1, HI):
            m1_c = m1pool.tile([NI, BG, OQ], BF16, tag="m1")
            m1_tiles.append(m1_c)
            for b in range(BG):
                if b >= 1:
                    t = b - 1
                    for ph in range(2):
                        mm2(ppsA[t], c - 1, 2 * t + ph, ph, m1_tiles[c - 1])
                mm1_b(x_t, m1_c, c, b)
            for ph in range(2):
                mm2(ppsA[3], c - 1, 6 + ph, ph, m1_tiles[c - 1])
        # wave-A mm2 for c=7 (dense)
        for t in range(4):
            for ph in range(2):
                mm2(ppsA[t], HI - 1, 2 * t + ph, ph, m1_tiles[HI - 1])

        evict_wave(0, ppsA[0:2], p_g)
        evict_wave(1, ppsA[2:4], p_g)

        # wave B: o-pairs 4..7, interleaved with next group's c=0 mm1 phase
        ppsB = [pps_alloc(f"ppsB_g{g}t{t}") for t in range(4)]
        if g + 1 < NG:
            m1_c0_carry = m1pool.tile([NI, BG, OQ], BF16, tag="m1")
            x_next = x_tiles[g + 1]
        else:
            m1_c0_carry = None
            x_next = None
        nb = 0  # next-group c0 batches emitted
        k = 0
        for t in range(4):
            for c in range(HI):
                for ph in range(2):
                    mm2(ppsB[t], c, 8 + 2 * t + ph, ph, m1_tiles[c])
                    k += 1
                    if x_next is not None and nb < BG and k % 8 == 0:
                        mm1_b(x_next, m1_c0_carry, 0, nb)
                        nb += 1

        evict_wave(2, ppsB[0:2], p_g)
        evict_wave(3, ppsB[2:4], p_g)

        # one 1 MB DMA for the whole group's P
        nc.scalar.dma_start(out=p_out[g], in_=p_g[:])


def prepare_inputs(x: np.ndarray, W: np.ndarray):
    """Full fp32 inputs -> per-core in_maps (bf16, device layouts)."""
    import ml_dtypes

    xb = x.astype(ml_dtypes.bfloat16)
    # [B_TOT, HI, NI(i), NI(j)] -> per core [NG, BG, HI, i, j] -> [NG, j, BG, HI, i]
    # (x symmetric in (i, j): partition dim reads as j, free as i)
    xb = xb.reshape(NCORES, NG, BG, HI, NI, NI).transpose(0, 1, 5, 2, 3, 4)
    xb = np.ascontiguousarray(xb)
    # W [HO, HI, NI, NO] -> [NI, HI, HO, NO]
    wb = np.ascontiguousarray(W.astype(ml_dtypes.bfloat16).transpose(2, 1, 0, 3))
    return [{"x": xb[i], "W": wb} for i in range(NCORES)]


def postprocess(results) -> np.ndarray:
    """Per-core P_dev [NG, (ph p), WV, T, BG, NO] -> full [B_TOT, HO, NO, NO]."""
    outs = []
    for i in range(NCORES):
        pd = results[i]["P"].reshape(NG, 2, NO, WV, 2, BG, NO)
        # o = 4*wv + 2*t + ph ; dims: g, ph, p, wv, t, b, q -> b(g,bg), o(wv,t,ph), p, q
        pd = pd.transpose(0, 5, 3, 4, 1, 2, 6).reshape(B, HO, NO, NO)
        outs.append(pd)
    return np.concatenate(outs, axis=0)


def kernel(x: np.ndarray, W: np.ndarray) -> np.ndarray:
    from concourse.bass_utils import run_bass_kernel_spmd

    in_maps = prepare_inputs(np.asarray(x, dtype=np.float32), np.asarray(W, dtype=np.float32))

    if "nc" not in _NC_CACHE:
        _NC_CACHE["nc"] = build_nc()
    nc = _NC_CACHE["nc"]

    res = run_bass_kernel_spmd(nc, in_maps, list(range(NCORES)))
    return postprocess(res.results)
